# revision 18
# baseline (speedup 1.0000x reference)
"""Multi-head self-attention (diag-zero mask) TRN2 kernel, 8-core head-parallel.

Sharding: 16 heads / 8 cores = 2 heads per core; every core sees the full
sequence (both batches), computes Q/K/V projections for its 2 heads,
attention, and its partial out-projection (Wo rows for its head block).
Host sums the 8 partial outputs (the out_proj all-reduce) and adds biases.

Math notes:
  - 1/sqrt(Dh) folded into Wq/bq on host.
  - scores are computed transposed (keys on partitions, queries on free dim)
    so exp() needs no on-chip reduction. The two heads' score matmuls have
    K=64 contraction on partition halves 0-63 / 64-127, so they run
    row-group-tiled *concurrently* on the PE.
  - V is projected directly into [keys, dims] layout (X chunk stationary,
    Wv moving) - no DMA transposes.
  - the A@V stationary is [V_h (64 cols) | ones (64 cols)]: rows 0-63 of the
    PSUM accumulator are the context, rows 64-127 are 64 copies of the
    softmax denominator Z. The extra ones columns are free (the matmul
    streams N=512 regardless) and give a full-width 64-lane reciprocal +
    multiply for the normalization (no 1-lane reciprocal, no broadcast).
  - diag-zero mask: multiply the score diagonal block by (1-eye) before exp
    (masked score 0 -> exp(0) = 1, matching the reference softmax).
  - bv and bo contributions are rank-1/constant terms folded in on host:
    out += bv @ Wo.T + bo.

Scheduling: the ACT engine's exp stream (~1.1us per key-tile) is the
bottleneck; the PE needs only ~0.65us per key-tile for scores + A@V. To keep
the PE dense (HAM stays un-throttled) and hide all other matmul work, the
batch-1 projections and the out-projections are chopped into small tasks and
pumped into the attention loops' idle slots.
"""

from contextlib import ExitStack

import numpy as np
import ml_dtypes

import concourse.bass as bass
import concourse.tile as tile
from concourse import bacc, mybir
from concourse.bass_utils import run_bass_kernel_spmd

BF16 = mybir.dt.bfloat16
F32 = mybir.dt.float32
FP8 = mybir.dt.float8e4
LN16 = 2.772588722239781

B = 2
D = 1024
H = 16
DH = 64
NCORES = 8
HLOC = H // NCORES          # 2 heads per core
DLOC = HLOC * DH            # 128 head-dims per core
KC = D // 128               # 8 contraction chunks for projections
MMW = 512                   # matmul moving width (one PSUM bank of f32)

import os
PUMP = os.environ.get("KERNEL_NO_PUMP", "") == ""   # filler interleaving on/off


def emit_kernel(tc, M, xT, wqT, wkT, wvT, woT, bq, bk, mask, out, dbg=None):
    """Emit the per-core program. M = per-batch sequence length."""
    nc = tc.nc
    S = B * M               # flattened sequence rows
    NKT = M // 128          # key tiles per batch
    NQT = M // MMW          # 512-wide q tiles per batch

    with ExitStack() as ctx:
        consts = ctx.enter_context(tc.tile_pool(name="consts", bufs=1))
        # per-batch tensors (separate tiles -> no false cross-batch deps)
        QT = [consts.tile([128, M], BF16, name=f"QT{b}") for b in range(B)]
        KT = [consts.tile([128, M], BF16, name=f"KT{b}") for b in range(B)]
        # A@V stationary: [V_h (64 cols) | ones (64 cols)] per (key-tile, head)
        V1 = [consts.tile([128, NKT, HLOC, 128], BF16, name=f"V1{b}")
              for b in range(B)]
        X = [consts.tile([128, KC, M], BF16, name=f"X{b}") for b in range(B)]
        C = [consts.tile([128, M], BF16, name=f"C{b}") for b in range(B)]
        Wq_sb = consts.tile([128, KC, DLOC], BF16)
        Wk_sb = consts.tile([128, KC, DLOC], BF16)
        Wv_sb = consts.tile([128, KC, DLOC], BF16)
        Wo_sb = consts.tile([128, D], BF16)
        bq_sb = consts.tile([128, 1], F32)
        bk_sb = consts.tile([128, 1], F32)
        mask_sb = consts.tile([128, 128], F32)
        mask2_sb_flat = consts.tile([128, HLOC * 128], F32)
        mask2_sb = mask2_sb_flat.rearrange("p (h q) -> p h q", h=HLOC)

        # weights first (they gate the first projection matmuls), then x in
        # chunks so the first m-tile's matmuls start early
        nc.sync.dma_start(Wk_sb, wkT.ap().rearrange("(c p) d -> p c d", p=128))
        nc.sync.dma_start(Wv_sb, wvT.ap().rearrange("(c p) d -> p c d", p=128))
        nc.sync.dma_start(Wq_sb, wqT.ap().rearrange("(c p) d -> p c d", p=128))
        nc.sync.dma_start(Wo_sb, woT.ap())
        nc.sync.dma_start(bq_sb, bq.ap())
        nc.sync.dma_start(bk_sb, bk.ap())
        nc.sync.dma_start(mask_sb, mask.ap())
        xT_r = xT.ap().rearrange("(c p) m -> p c m", p=128)
        for b in range(B):
            for mt in range(M // MMW):
                nc.sync.dma_start(
                    X[b][:, :, mt * MMW:(mt + 1) * MMW],
                    xT_r[:, :, b * M + mt * MMW: b * M + (mt + 1) * MMW],
                )
        for h in range(HLOC):
            nc.vector.tensor_copy(mask2_sb_flat[:, h * 128:(h + 1) * 128],
                                  mask_sb)
        for b in range(B):
            nc.vector.memset(V1[b][:, :, :, 64:128], 1.0)

        # PSUM pools: st 2x2 banks + ct 2x1 + pp 2x1 = 8 banks exactly
        stp = ctx.enter_context(tc.tile_pool(name="st_psum", bufs=2, space="PSUM"))
        ctp = ctx.enter_context(tc.tile_pool(name="ct_psum", bufs=2, space="PSUM"))
        pp = ctx.enter_context(tc.tile_pool(name="pp_psum", bufs=2, space="PSUM"))
        atp = ctx.enter_context(tc.tile_pool(name="at_pool", bufs=8))
        rzp = ctx.enter_context(tc.tile_pool(name="rz_pool", bufs=4))
        osp = ctx.enter_context(tc.tile_pool(name="out_sbuf", bufs=4))

        # HAM warm-up: ~16 dense matmuls on one stationary flip the PE clock
        # gate to 8/8 before the projections start. A cold start otherwise
        # sticks at half clock for the whole first phase: the ~93%-busy
        # production stream never shows the activity monitor one fully-busy
        # window, so it never un-throttles. Gated only on the small Wk DMA;
        # overlaps the (larger) X DMAs.
        wu = pp.tile([128, MMW], F32, name="fill_ps")
        wu_rhs = Wk_sb.rearrange("p c d -> p (c d)")[:, 0:MMW]
        for _ in range(8):
            nc.tensor.matmul(wu, lhsT=Wk_sb[:, 0, :], rhs=wu_rhs,
                             start=True, stop=True)

        # ---------------- task emitters ----------------
        def qk_proj_half(b, W_sb, dst, bias_sb, mt, half, ps_box):
            """Half of one 512-wide Q/K projection m-tile (4 of 8 kc steps)."""
            if half == 0:
                ps_box[0] = pp.tile([128, MMW], F32, name="fill_ps")
            ps = ps_box[0]
            for kc in range(half * 4, half * 4 + 4):
                nc.tensor.matmul(
                    ps,
                    lhsT=W_sb[:, kc, :],
                    rhs=X[b][:, kc, mt * MMW:(mt + 1) * MMW],
                    start=(kc == 0),
                    stop=(kc == KC - 1),
                )
            if half == 1:
                dslice = dst[:, mt * MMW:(mt + 1) * MMW]
                nc.vector.tensor_scalar_add(dslice, ps, bias_sb)

        def v_proj_mt(b, t):
            """One 128-wide V-projection tile, produced directly transposed:
            out[m, dloc] = sum_kc x[m, kc-block] @ WvT[kc-block, dloc]."""
            ps = pp.tile([128, MMW], F32, name="fill_ps")
            vp = ps[:, 0:DLOC]
            for kc in range(KC):
                nc.tensor.matmul(
                    vp,
                    lhsT=X[b][:, kc, t * 128:(t + 1) * 128],
                    rhs=Wv_sb[:, kc, :],
                    start=(kc == 0),
                    stop=(kc == KC - 1),
                )
            for h in range(HLOC):
                nc.vector.tensor_copy(V1[b][:, t, h, 0:64],
                                      vp[:, h * 64:(h + 1) * 64])

        def outproj_task(b, mt, j, eng="dve"):
            op = pp.tile([128, MMW], F32, name="fill_ps")
            nc.tensor.matmul(
                op,
                lhsT=C[b][:, mt * 128:(mt + 1) * 128],
                rhs=Wo_sb[:, j * MMW:(j + 1) * MMW],
                start=True,
                stop=True,
            )
            osb = osp.tile([128, MMW], BF16, name="osb")
            if eng == "act":
                # tail: ACT is idle after the last exp — split the PSUM
                # evacuations across both engines
                nc.scalar.activation(osb, op,
                                     mybir.ActivationFunctionType.Copy)
            else:
                nc.vector.tensor_copy(osb, op)
            nc.sync.dma_start(
                out.ap()[b * M + mt * 128: b * M + (mt + 1) * 128,
                         j * MMW:(j + 1) * MMW],
                osb,
            )

        def proj_tasks(b):
            """Filler tasks (est_pe_ns, fn, tag) for batch b's projection."""
            tasks = []
            for W_sb, dst, bias_sb in ((Wk_sb, KT[b], bk_sb),
                                       (Wq_sb, QT[b], bq_sb)):
                for mt in range(NQT):
                    box = [None]
                    for half in range(2):
                        tasks.append((900, (lambda b_=b, W=W_sb, d=dst,
                                            bb=bias_sb, m=mt, hf=half, bx=box:
                                            qk_proj_half(b_, W, d, bb, m, hf, bx)),
                                      "proj"))
            for t in range(NKT):
                tasks.append((800, (lambda b_=b, t_=t: v_proj_mt(b_, t_)), "proj"))
            return tasks

        def outproj_tasks(b, qt):
            # the final q-tile's evacuations run after the last exp: alternate
            # them onto the idle ACT engine
            tail = b == B - 1 and qt == NQT - 1
            return [(260, (lambda b_=b, m=mt, j_=j,
                           e=("act" if tail and (mt + j) % 2 else "dve"):
                           outproj_task(b_, m, j_, e)), "op")
                    for mt in range(qt * (MMW // 128), (qt + 1) * (MMW // 128))
                    for j in range(D // MMW)]

        # ---------------- attention ----------------
        def attn_batch(b, fillers):
            """fillers: mutable list of (est_ns, fn) pumped into idle PE slots.
            Out-projection tasks for completed q-tiles are appended as we go."""
            fill_budget = [0.0]

            def pump(ns):
                if not PUMP:
                    return
                fill_budget[0] += ns
                while fillers and fill_budget[0] >= fillers[0][0]:
                    est, fn, _tag = fillers.pop(0)
                    fill_budget[0] -= est
                    fn()

            for qt in range(NQT):
                q0 = qt * MMW
                at_tiles = []
                cts = [ctp.tile([128, MMW], F32, name="ct") for _ in range(HLOC)]

                def av(kt):
                    for h in range(HLOC):
                        nc.tensor.matmul(
                            cts[h],
                            lhsT=V1[b][:, kt, h, :],
                            rhs=at_tiles[kt][:, h * MMW:(h + 1) * MMW],
                            start=(kt == 0),
                            stop=(kt == NKT - 1),
                        )

                shift = min(3, NKT - 1)
                for kt in range(NKT):
                    # paired-head score tile: cols [0,512) = h0, [512,1024) = h1
                    # the two matmuls contract on partition halves 0-63/64-127
                    # -> row-group tiled, they run concurrently on the PE
                    st = stp.tile([128, HLOC * MMW], F32, name="st")
                    for h in range(HLOC):
                        hs = slice(h * 64, (h + 1) * 64)
                        nc.tensor.matmul(
                            st[:, h * MMW:(h + 1) * MMW],
                            lhsT=KT[b][hs, kt * 128:(kt + 1) * 128],
                            rhs=QT[b][hs, q0:q0 + MMW],
                            start=True,
                            stop=True,
                        )
                    c0 = kt * 128 - qt * MMW
                    if 0 <= c0 < MMW:
                        stv = st.rearrange("p (h q) -> p h q", h=HLOC)[:, :, c0:c0 + 128]
                        nc.vector.tensor_mul(stv, stv, mask2_sb)
                    at = atp.tile([128, HLOC * MMW], BF16, name="at")
                    nc.scalar.activation(at, st, mybir.ActivationFunctionType.Exp)
                    at_tiles.append(at)
                    if kt >= shift:
                        av(kt - shift)
                    pump(500)
                for kt in range(NKT - shift, NKT):
                    av(kt)
                # normalize: rows 64-127 of ct are 64 copies of Z. The custom
                # recip op reads garbage from PSUM, so stage Z through SBUF
                # with a plain copy first.
                for h, ct in enumerate(cts):
                    zsb = rzp.tile([64, MMW], F32, name="zsb")
                    nc.vector.tensor_copy(zsb, ct[64:128, :])
                    rz = rzp.tile([64, MMW], F32, name="rz")
                    nc.vector.reciprocal_approx_fast(rz, zsb)
                    nc.vector.tensor_mul(
                        C[b][h * 64:(h + 1) * 64, q0:q0 + MMW],
                        ct[0:64, :], rz,
                    )
                fillers.extend(outproj_tasks(b, qt))

        # ---------------- phases ----------------
        # batch-0 projection, dense
        for b0task in proj_tasks(0):
            b0task[1]()
        fillers = proj_tasks(1)
        attn_batch(0, fillers)       # pumps proj(b1) + outproj(b0)
        # attn(b1) reads QT[1]/KT[1]/V1[1]: every proj task must be emitted
        # before its first score matmul (proj tasks are a queue prefix)
        while fillers and fillers[0][2] == "proj":
            fillers.pop(0)[1]()
        attn_batch(1, fillers)       # pumps outproj leftovers + outproj(b1)
        for _, fn, _tag in fillers:  # drain the tail
            fn()

        if dbg is not None:
            for b in range(B):
                nc.sync.dma_start(dbg["qt"].ap()[:, b * M:(b + 1) * M], QT[b])
                nc.sync.dma_start(dbg["kt"].ap()[:, b * M:(b + 1) * M], KT[b])
                nc.sync.dma_start(
                    dbg["v1"].ap()[:, b * NKT * HLOC * 128:(b + 1) * NKT * HLOC * 128],
                    V1[b].rearrange("p a b c -> p (a b c)"))
                nc.sync.dma_start(dbg["c"].ap()[:, b * M:(b + 1) * M], C[b])


def build_bass(M, debug=False, reps=1):
    """Build + compile the per-core Bass program (same program on all cores).

    reps > 1 wraps the whole body in an on-device loop — used only for
    timing (amortizes host dispatch overhead over many executions).
    """
    S = B * M
    nc = bacc.Bacc("TRN2", target_bir_lowering=False, debug=False)
    xT = nc.dram_tensor("xT", [D, S], BF16, kind="ExternalInput")
    wqT = nc.dram_tensor("wqT", [D, DLOC], BF16, kind="ExternalInput")
    wkT = nc.dram_tensor("wkT", [D, DLOC], BF16, kind="ExternalInput")
    wvT = nc.dram_tensor("wvT", [D, DLOC], BF16, kind="ExternalInput")
    woT = nc.dram_tensor("woT", [DLOC, D], BF16, kind="ExternalInput")
    bq = nc.dram_tensor("bq", [DLOC, 1], F32, kind="ExternalInput")
    bk = nc.dram_tensor("bk", [DLOC, 1], F32, kind="ExternalInput")
    mask = nc.dram_tensor("mask", [128, 128], F32, kind="ExternalInput")
    out = nc.dram_tensor("out", [S, D], BF16, kind="ExternalOutput")

    dbg = None
    if debug:
        NKT = M // 128
        dbg = {
            "qt": nc.dram_tensor("dbg_qt", [128, S], BF16, kind="ExternalOutput"),
            "kt": nc.dram_tensor("dbg_kt", [128, S], BF16, kind="ExternalOutput"),
            "v1": nc.dram_tensor("dbg_v1", [128, B * NKT * HLOC * 128], BF16,
                                 kind="ExternalOutput"),
            "c": nc.dram_tensor("dbg_c", [128, S], BF16, kind="ExternalOutput"),
        }

    with tile.TileContext(nc) as tc:
        if reps > 1:
            # staggered_reset: overlap the loop's semaphore resets with
            # compute instead of the default drain + two all-engine barriers
            # per back-edge; hint_engines arms the branch-target prefetch
            # (the body far exceeds one IRAM block on every engine).
            ET = mybir.EngineType
            with tc.For_i(0, reps, 1, staggered_reset=True,
                          hint_engines=(ET.PE, ET.Activation, ET.DVE,
                                        ET.Pool, ET.SP)):
                emit_kernel(tc, M, xT, wqT, wkT, wvT, woT, bq, bk, mask, out,
                            dbg=dbg)
        else:
            emit_kernel(tc, M, xT, wqT, wkT, wvT, woT, bq, bk, mask, out,
                        dbg=dbg)
    nc.compile()
    return nc


def make_in_maps(M, x, Wq, bq, Wk, bk, Wv, Wo):
    """Host-side sharding: per-core input dicts."""
    S = B * M
    bf = ml_dtypes.bfloat16
    scale = 1.0 / np.sqrt(DH)
    xT = np.ascontiguousarray(x.reshape(S, D).T).astype(bf)
    mask = (1.0 - np.eye(128, dtype=np.float32))
    in_maps = []
    for c in range(NCORES):
        sl = slice(c * DLOC, (c + 1) * DLOC)
        in_maps.append({
            "xT": xT,
            "wqT": np.ascontiguousarray((Wq[sl] * scale).T).astype(bf),
            "wkT": np.ascontiguousarray(Wk[sl].T).astype(bf),
            "wvT": np.ascontiguousarray(Wv[sl].T).astype(bf),
            "woT": np.ascontiguousarray(Wo[:, sl].T).astype(bf),
            "bq": (bq[sl] * scale).reshape(DLOC, 1).astype(np.float32),
            "bk": bk[sl].reshape(DLOC, 1).astype(np.float32),
            "mask": mask,
        })
    return in_maps


_NC_CACHE = {}


def kernel(x, Wq, bq, Wk, bk, Wv, bv, Wo, bo):
    x = np.asarray(x, dtype=np.float32)
    Wq = np.asarray(Wq, dtype=np.float32)
    bq = np.asarray(bq, dtype=np.float32)
    Wk = np.asarray(Wk, dtype=np.float32)
    bk = np.asarray(bk, dtype=np.float32)
    Wv = np.asarray(Wv, dtype=np.float32)
    bv = np.asarray(bv, dtype=np.float32)
    Wo = np.asarray(Wo, dtype=np.float32)
    bo = np.asarray(bo, dtype=np.float32)

    M = x.shape[1]
    if M not in _NC_CACHE:
        _NC_CACHE[M] = build_bass(M)
    nc = _NC_CACHE[M]

    in_maps = make_in_maps(M, x, Wq, bq, Wk, bk, Wv, Wo)
    res = run_bass_kernel_spmd(nc, in_maps, core_ids=list(range(NCORES)))

    out = np.zeros((B * M, D), np.float64)
    for c in range(NCORES):
        out += res.results[c]["out"].astype(np.float64)
    out = out.astype(np.float32)
    out += bv @ Wo.T + bo          # folded bv/bo contribution
    return out.reshape(B, M, D)
